# revision 28
# baseline (speedup 1.0000x reference)
"""Trainium2 Bass kernel for nn_LinearLatentKernel_84834194031187.

Computes, for x:[B,S,D], W_qkv:[3D,D], W_gate:[D,D] (fp32):
    qkv = x @ W_qkv.T + b_qkv ; q,k,v = split(qkv)
    kv_state = cumsum(k*v, axis=seq)
    out = q * kv_state * sigmoid(x @ W_gate.T + b_gate)

Sharding: 8 cores = (batch b in 0..3) x (channel half h in 0..1). Each core
handles x[b] [S,D] against host-pretransposed weight slices and produces
out[b,:,h*H:(h+1)*H].

Per core, seq is processed in 32 blocks of 128 rows (partition dim = seq):
  - x^T tiles are prepared on the HOST (permuted so each block's 8 [d=128,
    s=128] contraction tiles load with one contiguous-line DMA); no PE
    transposes on device.
  - k/v/q chunks [128, 512] accumulate in PSUM over 8 bf16 contraction
    tiles; the GATE runs in fp8(e4m3) DoubleRow mode (2 contraction
    elements per PE cell -> 4 matmuls instead of 8). The gate only passes
    through a sigmoid, so fp8 error there is strongly attenuated; gate
    weights are pre-scaled x32 on the host (undone by the sigmoid's scale)
    to clear e4m3's subnormal floor.
  - kv = k*v; block-cumsum via a single matmul with an upper-triangular
    ones lhsT, threaded mid-q-group so its inputs are ready early. The
    running carry never touches the PE: a GpSimd partition_all_reduce
    produces colsum(kv) broadcast to all partitions, vector adds accumulate
    and apply it.
  - out = (q * sigmoid(g)) * kv_state, streamed back to DRAM.
  - input DMAs alternate between both HWDGE rings (Sync + Scalar) in
    matmul-consumption order so the weight preload streams at HBM rate.
"""

import numpy as np

import concourse.bass as bass
import concourse.bacc as bacc
import concourse.tile as tile
import concourse.mybir as mybir
from concourse import bass_isa
from concourse.bass_utils import run_bass_kernel_spmd

B, S, D = 4, 4096, 1024
H = 512          # channels per core (half of D)
P = 128
NBLK = S // P    # 32 seq blocks
KT = D // P      # 8 contraction tiles
NWARM = 48       # dummy warmup matmuls to keep HAM busy during initial DMA
GSCALE = 32.0    # host pre-scale on gate weights (undone in the sigmoid)

f32 = mybir.dt.float32
f32r = mybir.dt.float32r
bf16 = mybir.dt.bfloat16
f16 = mybir.dt.float16
f8e4 = mybir.dt.float8e4

PROJ_DT = "bf16"

_NC_CACHE = {}

# bf16 channel-group order inside wt: k, v (kv feeds the cumsum) then q
CK, CV, CQ = 0, 1, 2


def _build(with_bias: bool, proj: str = PROJ_DT):
    proj_dt = {"f16": f16, "bf16": bf16, "f32r": f32r}[proj]
    nc = bacc.Bacc("TRN2", target_bir_lowering=False)

    # host-permuted x^T: row i*128+p, col kt*128+s  ==  x[i*128+s, kt*128+p]
    xt_d = nc.dram_tensor("xt", [S, D], proj_dt, kind="ExternalInput")
    xt8_d = nc.dram_tensor("xt8", [S, D], f8e4, kind="ExternalInput")
    # host-packed bf16 weights (k,v,q): row (c*4+j)*128+p, col kt'*512+col
    # holds wt[(2*j+kt')*128+p, c*512+col] — one contiguous unit per
    # (channel-group c, kt-pair j), in matmul-consumption order
    wtp_d = nc.dram_tensor("wtp", [12 * P, 2 * H], proj_dt, kind="ExternalInput")
    # gate weights (x GSCALE) in fp8, packed per kt-quad unit u:
    # row u*128+p, col kt'*512+col holds wg[(u*4+kt')*128+p, col]
    wtg8_d = nc.dram_tensor("wtg8", [2 * P, 4 * H], f8e4, kind="ExternalInput")
    tri_d = nc.dram_tensor("tri", [P, P], bf16, kind="ExternalInput")
    if with_bias:
        onesrow_d = nc.dram_tensor("onesrow", [1, P], f32r, kind="ExternalInput")
        bias_d = nc.dram_tensor("bias", [1, 4 * H], f32r, kind="ExternalInput")
    out_d = nc.dram_tensor("out", [S, H], bf16, kind="ExternalOutput")

    with tile.TileContext(nc) as tc:
        with (
            tc.tile_pool(name="consts", bufs=1) as consts,
            tc.tile_pool(name="xin", bufs=6) as xin,
            tc.tile_pool(name="work", bufs=2) as work,
            tc.tile_pool(name="redp", bufs=3) as redp,
            tc.tile_pool(name="outp", bufs=3) as outp,
            tc.tile_pool(name="pmm", bufs=1, space="PSUM") as pmm,
            tc.tile_pool(name="pwarm", bufs=1, space="PSUM") as pwarm,
            tc.tile_pool(name="pcs_pool", bufs=2, space="PSUM") as pcs_pool,
        ):
            # ---- PE warmup: dummy matmuls on a scratch tile (result never
            # read) so the HAM clock-gate opens while the first DMAs are in
            # flight.
            warm_sb = consts.tile([P, P], proj_dt, tag="warm")
            nc.vector.memset(warm_sb[:], 0.0)
            pw = pwarm.tile([P, P], f32, tag="pw", name="pw")
            for w in range(NWARM):
                nc.tensor.matmul(pw[:], warm_sb[:], warm_sb[:],
                                 start=True, stop=True)

            # ---- input DMAs, split across BOTH HWDGE rings in consumption
            # order: x blocks + gate fp8 first, then k,v,q bf16 units.
            xb8_0 = xin.tile([P, KT, P], f8e4, tag="xb8", name="xb8_0")
            nc.sync.dma_start(xb8_0[:], xt8_d[0:P, :])
            xb0 = xin.tile([P, KT * P], proj_dt, tag="xb", name="xb0")
            nc.scalar.dma_start(xb0[:], xt_d[0:P, :])
            wtg8_sb = consts.tile([P, KT, H], f8e4, tag="wtg8")
            nc.sync.dma_start(wtg8_sb[:, 0:4, :], wtg8_d[0:P, :])
            nc.scalar.dma_start(wtg8_sb[:, 4:KT, :], wtg8_d[P:2 * P, :])
            tri_sb = consts.tile([P, P], bf16, tag="tri")
            nc.sync.dma_start(tri_sb[:], tri_d[:])
            wt_sb = consts.tile([P, KT, 3 * H], proj_dt, tag="wt")
            for c in range(3):
                for j in range(4):
                    ring = nc.sync if (j % 2 == 0) else nc.scalar
                    r = (c * 4 + j) * P
                    ring.dma_start(
                        wt_sb[:, 2 * j:2 * j + 2, c * H:(c + 1) * H],
                        wtp_d[r:r + P, :])
            if with_bias:
                onesrow_sb = consts.tile([1, P], f32r, tag="onesrow")
                nc.scalar.dma_start(onesrow_sb[:], onesrow_d[:])
                bias_sb = consts.tile([1, 4 * H], f32r, tag="bias")
                nc.scalar.dma_start(bias_sb[:], bias_d[:])

            def warm_fill(n):
                # filler matmuls with no DMA dependency: execute only while
                # the PE would otherwise idle waiting for a weight unit,
                # keeping the HAM activity monitor from re-throttling
                for _ in range(n):
                    nc.tensor.matmul(pw[:], warm_sb[:], warm_sb[:],
                                     start=True, stop=True)

            def mm_group(ps_c, xb, c, kts, start, stop, bias_c=None):
                for kt in kts:
                    nc.tensor.matmul(
                        ps_c[:], xb[:, kt * P:(kt + 1) * P],
                        wt_sb[:, kt, c * H:(c + 1) * H],
                        start=(start and kt == kts[0]),
                        stop=(stop and kt == kts[-1] and not with_bias),
                    )
                if stop and with_bias:
                    nc.tensor.matmul(
                        ps_c[:], onesrow_sb[:], bias_sb[:, bias_c * H:(bias_c + 1) * H],
                        start=False, stop=True,
                    )

            bcc = None  # broadcast running carry [128,H], None for block 0
            for i in range(NBLK):
                last = (i == NBLK - 1)
                if i == 0:
                    xb, xb8 = xb0, xb8_0
                else:
                    xb8 = xin.tile([P, KT, P], f8e4, tag="xb8")
                    nc.sync.dma_start(xb8[:], xt8_d[i * P:(i + 1) * P, :])
                    xb = xin.tile([P, KT * P], proj_dt, tag="xb")
                    nc.sync.dma_start(xb[:], xt_d[i * P:(i + 1) * P, :])

                psg = pmm.tile([P, H], f32, tag="psg", name="psg")
                ps = [pmm.tile([P, H], f32, tag=f"ps{c}", name=f"ps{c}")
                      for c in range(3)]

                # gate projection: fp8 DoubleRow over kt-pairs
                for j in range(4):
                    nc.tensor.matmul(
                        psg[:], xb8[:, 2 * j:2 * j + 2, :],
                        wtg8_sb[:, 2 * j:2 * j + 2, :],
                        start=(j == 0),
                        stop=(j == 3 and not with_bias),
                        perf_mode=mybir.MatmulPerfMode.DoubleRow,
                    )
                if with_bias:
                    nc.tensor.matmul(
                        psg[:], onesrow_sb[:], bias_sb[:, 3 * H:4 * H],
                        start=False, stop=True,
                    )
                for c in (CK, CV):
                    if i < 2:
                        warm_fill(6)
                    mm_group(ps[c], xb, c, list(range(KT)), True, True,
                             bias_c=c)

                g_sb = work.tile([P, H], f32, tag="g")
                nc.scalar.activation(
                    g_sb[:], psg[:], mybir.ActivationFunctionType.Sigmoid,
                    scale=1.0 / GSCALE)
                k_sb = work.tile([P, H], f32, tag="k")
                nc.any.tensor_copy(out=k_sb[:], in_=ps[CK][:])
                kv_sb = work.tile([P, H], bf16, tag="kv")
                nc.vector.tensor_mul(out=kv_sb[:], in0=k_sb[:], in1=ps[CV][:])

                # q-projection, with the cumsum matmul threaded mid-group so
                # its inputs are ready and the output chain starts early
                if i < 2:
                    warm_fill(6)
                mm_group(ps[CQ], xb, CQ, [0, 1, 2, 3], True, False)
                pcs = pcs_pool.tile([P, H], f32, tag="pcs")
                nc.tensor.matmul(pcs[:], tri_sb[:], kv_sb[:],
                                 start=True, stop=True)
                if bcc is not None:
                    state = work.tile([P, H], f32, tag="state")
                    nc.vector.tensor_add(out=state[:], in0=bcc[:], in1=pcs[:])
                else:
                    state = pcs
                mm_group(ps[CQ], xb, CQ, [4, 5, 6, 7], False, True, bias_c=CQ)

                qg_sb = work.tile([P, H], f32, tag="qg")
                nc.vector.tensor_mul(out=qg_sb[:], in0=g_sb[:], in1=ps[CQ][:])
                ob = outp.tile([P, H], bf16, tag="ob")
                nc.vector.tensor_mul(out=ob[:], in0=qg_sb[:], in1=state[:])
                # last block's store goes on the Scalar ring so it overlaps
                # the previous block's store instead of queueing behind it
                ring = nc.scalar if last else nc.sync
                ring.dma_start(out_d[i * P:(i + 1) * P, :], ob[:])

                if not last:
                    # colsum(kv_i) broadcast to all partitions (GpSimd), then
                    # fold into the running broadcast carry for block i+1
                    red = redp.tile([P, H], f32, tag="red")
                    nc.gpsimd.partition_all_reduce(
                        red[:], kv_sb[:], channels=P,
                        reduce_op=bass_isa.ReduceOp.add)
                    if bcc is None:
                        bcc = red
                    else:
                        nbcc = redp.tile([P, H], f32, tag="bcc")
                        nc.vector.tensor_add(out=nbcc[:], in0=bcc[:], in1=red[:])
                        bcc = nbcc

    nc.compile()
    return nc


def _get_nc(with_bias: bool):
    if with_bias not in _NC_CACHE:
        _NC_CACHE[with_bias] = _build(with_bias)
    return _NC_CACHE[with_bias]


def _proj_np_dtype():
    if PROJ_DT == "bf16":
        import ml_dtypes
        return ml_dtypes.bfloat16
    if PROJ_DT == "f16":
        return np.float16
    return np.float32


def _perm(x2d):
    x3 = x2d.reshape(NBLK, P, KT, P).transpose(0, 3, 2, 1)
    return np.ascontiguousarray(x3).reshape(S, D)


def _prep_in_maps(x, W_qkv, b_qkv, W_gate, b_gate, with_bias):
    import ml_dtypes
    e4 = ml_dtypes.float8_e4m3fn
    pdt = _proj_np_dtype()
    x = np.ascontiguousarray(np.asarray(x, dtype=np.float32))
    W_qkv = np.asarray(W_qkv, dtype=np.float32)
    W_gate = np.asarray(W_gate, dtype=np.float32)

    # device layout: row i*128+p, col kt*128+s == x[b][i*128+s, kt*128+p]
    xts = [_perm(x[b].astype(pdt)) for b in range(B)]
    x8s = [_perm(np.clip(x[b], -240, 240).astype(e4)) for b in range(B)]

    consts = {
        "tri": np.triu(np.ones((P, P), dtype=np.float32)).astype(ml_dtypes.bfloat16),
    }
    if with_bias:
        consts["onesrow"] = np.ones((1, P), dtype=np.float32)

    wts, wg8s, biases = [], [], []
    for h in range(2):
        sl = slice(h * H, (h + 1) * H)
        # bf16 channel-group order: k, v, q
        wt = np.concatenate(
            [W_qkv[D + h * H:D + (h + 1) * H],
             W_qkv[2 * D + h * H:2 * D + (h + 1) * H], W_qkv[sl]], axis=0
        ).T.astype(pdt)
        # pack into per-(c, kt-pair) contiguous units:
        # wtp[(c*4+j)*128+p, kt'*512+col] = wt[(2j+kt')*128+p, c*512+col]
        wtp = wt.reshape(4, 2, P, 3, H).transpose(3, 0, 2, 1, 4)
        wts.append(np.ascontiguousarray(wtp).reshape(12 * P, 2 * H))
        wg = W_gate[sl].T * GSCALE  # [D, H]
        wg8 = np.clip(wg, -240, 240).astype(e4)
        # pack per kt-quad unit: wtg8[u*128+p, kt'*512+col] = wg[(u*4+kt')*128+p, col]
        wg8 = wg8.reshape(2, 4, P, H).transpose(0, 2, 1, 3)
        wg8s.append(np.ascontiguousarray(wg8).reshape(2 * P, 4 * H))
        if with_bias:
            bq = np.asarray(b_qkv, dtype=np.float32)
            bg = np.asarray(b_gate, dtype=np.float32) * GSCALE
            biases.append(np.concatenate(
                [bq[D + h * H:D + (h + 1) * H],
                 bq[2 * D + h * H:2 * D + (h + 1) * H], bq[sl], bg[sl]]
            )[None, :].copy())

    in_maps = []
    for core in range(8):
        b, h = core // 2, core % 2
        m = {"xt": xts[b], "xt8": x8s[b], "wtp": wts[h], "wtg8": wg8s[h],
             **consts}
        if with_bias:
            m["bias"] = biases[h]
        in_maps.append(m)
    return in_maps


def run(x, W_qkv, b_qkv, W_gate, b_gate, trace=False, **run_kwargs):
    with_bias = bool(np.any(np.asarray(b_qkv)) or np.any(np.asarray(b_gate)))
    nc = _get_nc(with_bias)
    in_maps = _prep_in_maps(x, W_qkv, b_qkv, W_gate, b_gate, with_bias)
    res = run_bass_kernel_spmd(nc, in_maps, list(range(8)), trace=trace, **run_kwargs)
    out = np.empty((B, S, D), dtype=np.float32)
    for core in range(8):
        b, h = core // 2, core % 2
        out[b, :, h * H:(h + 1) * H] = np.asarray(
            res.results[core]["out"], dtype=np.float32)
    return out, res


def kernel(x, W_qkv, b_qkv, W_gate, b_gate):
    out, _ = run(x, W_qkv, b_qkv, W_gate, b_gate)
    return out


# revision 29
# speedup vs baseline: 1.0022x; 1.0022x over previous
"""Trainium2 Bass kernel for nn_LinearLatentKernel_84834194031187.

Computes, for x:[B,S,D], W_qkv:[3D,D], W_gate:[D,D] (fp32):
    qkv = x @ W_qkv.T + b_qkv ; q,k,v = split(qkv)
    kv_state = cumsum(k*v, axis=seq)
    out = q * kv_state * sigmoid(x @ W_gate.T + b_gate)

Sharding: 8 cores = (batch b in 0..3) x (channel half h in 0..1). Each core
handles x[b] [S,D] against host-pretransposed weight slices and produces
out[b,:,h*H:(h+1)*H].

Per core, seq is processed in 32 blocks of 128 rows (partition dim = seq):
  - x^T tiles are prepared on the HOST (permuted so each block's 8 [d=128,
    s=128] contraction tiles load with one contiguous-line DMA); no PE
    transposes on device.
  - k/v/q chunks [128, 512] accumulate in PSUM over 8 bf16 contraction
    tiles; the GATE runs in fp8(e4m3) DoubleRow mode (2 contraction
    elements per PE cell -> 4 matmuls instead of 8). The gate only passes
    through a sigmoid, so fp8 error there is strongly attenuated; gate
    weights are pre-scaled x32 on the host (undone by the sigmoid's scale)
    to clear e4m3's subnormal floor.
  - kv = k*v; block-cumsum via a single matmul with an upper-triangular
    ones lhsT, threaded mid-q-group so its inputs are ready early. The
    running carry never touches the PE: a GpSimd partition_all_reduce
    produces colsum(kv) broadcast to all partitions, vector adds accumulate
    and apply it.
  - out = (q * sigmoid(g)) * kv_state, streamed back to DRAM.
  - input DMAs alternate between both HWDGE rings (Sync + Scalar) in
    matmul-consumption order so the weight preload streams at HBM rate.
"""

import numpy as np

import concourse.bass as bass
import concourse.bacc as bacc
import concourse.tile as tile
import concourse.mybir as mybir
from concourse import bass_isa
from concourse.bass_utils import run_bass_kernel_spmd

B, S, D = 4, 4096, 1024
H = 512          # channels per core (half of D)
P = 128
NBLK = S // P    # 32 seq blocks
KT = D // P      # 8 contraction tiles
NWARM = 48       # dummy warmup matmuls to keep HAM busy during initial DMA
GSCALE = 32.0    # host pre-scale on gate weights (undone in the sigmoid)

f32 = mybir.dt.float32
f32r = mybir.dt.float32r
bf16 = mybir.dt.bfloat16
f16 = mybir.dt.float16
f8e4 = mybir.dt.float8e4

PROJ_DT = "bf16"

_NC_CACHE = {}

# bf16 channel-group order inside wt: k, v (kv feeds the cumsum) then q
CK, CV, CQ = 0, 1, 2


def _build(with_bias: bool, proj: str = PROJ_DT):
    proj_dt = {"f16": f16, "bf16": bf16, "f32r": f32r}[proj]
    nc = bacc.Bacc("TRN2", target_bir_lowering=False)

    # host-permuted x^T: row i*128+p, col kt*128+s  ==  x[i*128+s, kt*128+p]
    xt_d = nc.dram_tensor("xt", [S, D], proj_dt, kind="ExternalInput")
    xt8_d = nc.dram_tensor("xt8", [S, D], f8e4, kind="ExternalInput")
    # host-packed bf16 weights (k,v,q): row (c*4+j)*128+p, col kt'*512+col
    # holds wt[(2*j+kt')*128+p, c*512+col] — one contiguous unit per
    # (channel-group c, kt-pair j), in matmul-consumption order
    wtp_d = nc.dram_tensor("wtp", [12 * P, 2 * H], proj_dt, kind="ExternalInput")
    # gate weights (x GSCALE) in fp8, packed per kt-quad unit u:
    # row u*128+p, col kt'*512+col holds wg[(u*4+kt')*128+p, col]
    wtg8_d = nc.dram_tensor("wtg8", [2 * P, 4 * H], f8e4, kind="ExternalInput")
    tri_d = nc.dram_tensor("tri", [P, P], bf16, kind="ExternalInput")
    if with_bias:
        onesrow_d = nc.dram_tensor("onesrow", [1, P], f32r, kind="ExternalInput")
        bias_d = nc.dram_tensor("bias", [1, 4 * H], f32r, kind="ExternalInput")
    out_d = nc.dram_tensor("out", [S, H], bf16, kind="ExternalOutput")

    with tile.TileContext(nc) as tc:
        with (
            tc.tile_pool(name="consts", bufs=1) as consts,
            tc.tile_pool(name="xin", bufs=6) as xin,
            tc.tile_pool(name="work", bufs=2) as work,
            tc.tile_pool(name="redp", bufs=3) as redp,
            tc.tile_pool(name="outp", bufs=3) as outp,
            tc.tile_pool(name="pmm", bufs=1, space="PSUM") as pmm,
            tc.tile_pool(name="pwarm", bufs=1, space="PSUM") as pwarm,
            tc.tile_pool(name="pcs_pool", bufs=2, space="PSUM") as pcs_pool,
        ):
            # ---- PE warmup: dummy matmuls on a scratch tile (result never
            # read) so the HAM clock-gate opens while the first DMAs are in
            # flight.
            warm_sb = consts.tile([P, P], proj_dt, tag="warm")
            nc.vector.memset(warm_sb[:], 0.0)
            pw = pwarm.tile([P, P], f32, tag="pw", name="pw")
            for w in range(NWARM):
                nc.tensor.matmul(pw[:], warm_sb[:], warm_sb[:],
                                 start=True, stop=True)

            # ---- input DMAs, split across BOTH HWDGE rings in consumption
            # order: x blocks + gate fp8 first, then k,v,q bf16 units.
            xb8_0 = xin.tile([P, KT, P], f8e4, tag="xb8", name="xb8_0")
            nc.sync.dma_start(xb8_0[:], xt8_d[0:P, :])
            xb0 = xin.tile([P, KT * P], proj_dt, tag="xb", name="xb0")
            nc.scalar.dma_start(xb0[:], xt_d[0:P, :])
            wtg8_sb = consts.tile([P, KT, H], f8e4, tag="wtg8")
            nc.sync.dma_start(wtg8_sb[:, 0:4, :], wtg8_d[0:P, :])
            nc.scalar.dma_start(wtg8_sb[:, 4:KT, :], wtg8_d[P:2 * P, :])
            tri_sb = consts.tile([P, P], bf16, tag="tri")
            nc.sync.dma_start(tri_sb[:], tri_d[:])
            wt_sb = consts.tile([P, KT, 3 * H], proj_dt, tag="wt")
            for c in range(3):
                for j in range(4):
                    ring = nc.sync if (j % 2 == 0) else nc.scalar
                    r = (c * 4 + j) * P
                    ring.dma_start(
                        wt_sb[:, 2 * j:2 * j + 2, c * H:(c + 1) * H],
                        wtp_d[r:r + P, :])
            if with_bias:
                onesrow_sb = consts.tile([1, P], f32r, tag="onesrow")
                nc.scalar.dma_start(onesrow_sb[:], onesrow_d[:])
                bias_sb = consts.tile([1, 4 * H], f32r, tag="bias")
                nc.scalar.dma_start(bias_sb[:], bias_d[:])

            def mm_group(ps_c, xb, c, kts, start, stop, bias_c=None):
                for kt in kts:
                    nc.tensor.matmul(
                        ps_c[:], xb[:, kt * P:(kt + 1) * P],
                        wt_sb[:, kt, c * H:(c + 1) * H],
                        start=(start and kt == kts[0]),
                        stop=(stop and kt == kts[-1] and not with_bias),
                    )
                if stop and with_bias:
                    nc.tensor.matmul(
                        ps_c[:], onesrow_sb[:], bias_sb[:, bias_c * H:(bias_c + 1) * H],
                        start=False, stop=True,
                    )

            bcc = None  # broadcast running carry [128,H], None for block 0
            for i in range(NBLK):
                last = (i == NBLK - 1)
                if i == 0:
                    xb, xb8 = xb0, xb8_0
                else:
                    xb8 = xin.tile([P, KT, P], f8e4, tag="xb8")
                    nc.sync.dma_start(xb8[:], xt8_d[i * P:(i + 1) * P, :])
                    xb = xin.tile([P, KT * P], proj_dt, tag="xb")
                    nc.sync.dma_start(xb[:], xt_d[i * P:(i + 1) * P, :])

                psg = pmm.tile([P, H], f32, tag="psg", name="psg")
                ps = [pmm.tile([P, H], f32, tag=f"ps{c}", name=f"ps{c}")
                      for c in range(3)]

                # gate projection: fp8 DoubleRow over kt-pairs
                for j in range(4):
                    nc.tensor.matmul(
                        psg[:], xb8[:, 2 * j:2 * j + 2, :],
                        wtg8_sb[:, 2 * j:2 * j + 2, :],
                        start=(j == 0),
                        stop=(j == 3 and not with_bias),
                        perf_mode=mybir.MatmulPerfMode.DoubleRow,
                    )
                if with_bias:
                    nc.tensor.matmul(
                        psg[:], onesrow_sb[:], bias_sb[:, 3 * H:4 * H],
                        start=False, stop=True,
                    )
                for c in (CK, CV):
                    mm_group(ps[c], xb, c, list(range(KT)), True, True,
                             bias_c=c)

                g_sb = work.tile([P, H], f32, tag="g")
                nc.scalar.activation(
                    g_sb[:], psg[:], mybir.ActivationFunctionType.Sigmoid,
                    scale=1.0 / GSCALE)
                k_sb = work.tile([P, H], f32, tag="k")
                nc.any.tensor_copy(out=k_sb[:], in_=ps[CK][:])
                kv_sb = work.tile([P, H], bf16, tag="kv")
                nc.vector.tensor_mul(out=kv_sb[:], in0=k_sb[:], in1=ps[CV][:])

                # q-projection, with the cumsum matmul threaded mid-group so
                # its inputs are ready and the output chain starts early
                mm_group(ps[CQ], xb, CQ, [0, 1, 2, 3], True, False)
                pcs = pcs_pool.tile([P, H], f32, tag="pcs")
                nc.tensor.matmul(pcs[:], tri_sb[:], kv_sb[:],
                                 start=True, stop=True)
                if bcc is not None:
                    state = work.tile([P, H], f32, tag="state")
                    nc.vector.tensor_add(out=state[:], in0=bcc[:], in1=pcs[:])
                else:
                    state = pcs
                mm_group(ps[CQ], xb, CQ, [4, 5, 6, 7], False, True, bias_c=CQ)

                qg_sb = work.tile([P, H], f32, tag="qg")
                nc.vector.tensor_mul(out=qg_sb[:], in0=g_sb[:], in1=ps[CQ][:])
                ob = outp.tile([P, H], bf16, tag="ob")
                nc.vector.tensor_mul(out=ob[:], in0=qg_sb[:], in1=state[:])
                # last block's store goes on the Scalar ring so it overlaps
                # the previous block's store instead of queueing behind it
                ring = nc.scalar if last else nc.sync
                ring.dma_start(out_d[i * P:(i + 1) * P, :], ob[:])

                if not last:
                    # colsum(kv_i) broadcast to all partitions (GpSimd), then
                    # fold into the running broadcast carry for block i+1
                    red = redp.tile([P, H], f32, tag="red")
                    nc.gpsimd.partition_all_reduce(
                        red[:], kv_sb[:], channels=P,
                        reduce_op=bass_isa.ReduceOp.add)
                    if bcc is None:
                        bcc = red
                    else:
                        nbcc = redp.tile([P, H], f32, tag="bcc")
                        nc.vector.tensor_add(out=nbcc[:], in0=bcc[:], in1=red[:])
                        bcc = nbcc

    nc.compile()
    return nc


def _get_nc(with_bias: bool):
    if with_bias not in _NC_CACHE:
        _NC_CACHE[with_bias] = _build(with_bias)
    return _NC_CACHE[with_bias]


def _proj_np_dtype():
    if PROJ_DT == "bf16":
        import ml_dtypes
        return ml_dtypes.bfloat16
    if PROJ_DT == "f16":
        return np.float16
    return np.float32


def _perm(x2d):
    x3 = x2d.reshape(NBLK, P, KT, P).transpose(0, 3, 2, 1)
    return np.ascontiguousarray(x3).reshape(S, D)


def _prep_in_maps(x, W_qkv, b_qkv, W_gate, b_gate, with_bias):
    import ml_dtypes
    e4 = ml_dtypes.float8_e4m3fn
    pdt = _proj_np_dtype()
    x = np.ascontiguousarray(np.asarray(x, dtype=np.float32))
    W_qkv = np.asarray(W_qkv, dtype=np.float32)
    W_gate = np.asarray(W_gate, dtype=np.float32)

    # device layout: row i*128+p, col kt*128+s == x[b][i*128+s, kt*128+p]
    xts = [_perm(x[b].astype(pdt)) for b in range(B)]
    x8s = [_perm(np.clip(x[b], -240, 240).astype(e4)) for b in range(B)]

    consts = {
        "tri": np.triu(np.ones((P, P), dtype=np.float32)).astype(ml_dtypes.bfloat16),
    }
    if with_bias:
        consts["onesrow"] = np.ones((1, P), dtype=np.float32)

    wts, wg8s, biases = [], [], []
    for h in range(2):
        sl = slice(h * H, (h + 1) * H)
        # bf16 channel-group order: k, v, q
        wt = np.concatenate(
            [W_qkv[D + h * H:D + (h + 1) * H],
             W_qkv[2 * D + h * H:2 * D + (h + 1) * H], W_qkv[sl]], axis=0
        ).T.astype(pdt)
        # pack into per-(c, kt-pair) contiguous units:
        # wtp[(c*4+j)*128+p, kt'*512+col] = wt[(2j+kt')*128+p, c*512+col]
        wtp = wt.reshape(4, 2, P, 3, H).transpose(3, 0, 2, 1, 4)
        wts.append(np.ascontiguousarray(wtp).reshape(12 * P, 2 * H))
        wg = W_gate[sl].T * GSCALE  # [D, H]
        wg8 = np.clip(wg, -240, 240).astype(e4)
        # pack per kt-quad unit: wtg8[u*128+p, kt'*512+col] = wg[(u*4+kt')*128+p, col]
        wg8 = wg8.reshape(2, 4, P, H).transpose(0, 2, 1, 3)
        wg8s.append(np.ascontiguousarray(wg8).reshape(2 * P, 4 * H))
        if with_bias:
            bq = np.asarray(b_qkv, dtype=np.float32)
            bg = np.asarray(b_gate, dtype=np.float32) * GSCALE
            biases.append(np.concatenate(
                [bq[D + h * H:D + (h + 1) * H],
                 bq[2 * D + h * H:2 * D + (h + 1) * H], bq[sl], bg[sl]]
            )[None, :].copy())

    in_maps = []
    for core in range(8):
        b, h = core // 2, core % 2
        m = {"xt": xts[b], "xt8": x8s[b], "wtp": wts[h], "wtg8": wg8s[h],
             **consts}
        if with_bias:
            m["bias"] = biases[h]
        in_maps.append(m)
    return in_maps


def run(x, W_qkv, b_qkv, W_gate, b_gate, trace=False, **run_kwargs):
    with_bias = bool(np.any(np.asarray(b_qkv)) or np.any(np.asarray(b_gate)))
    nc = _get_nc(with_bias)
    in_maps = _prep_in_maps(x, W_qkv, b_qkv, W_gate, b_gate, with_bias)
    res = run_bass_kernel_spmd(nc, in_maps, list(range(8)), trace=trace, **run_kwargs)
    out = np.empty((B, S, D), dtype=np.float32)
    for core in range(8):
        b, h = core // 2, core % 2
        out[b, :, h * H:(h + 1) * H] = np.asarray(
            res.results[core]["out"], dtype=np.float32)
    return out, res


def kernel(x, W_qkv, b_qkv, W_gate, b_gate):
    out, _ = run(x, W_qkv, b_qkv, W_gate, b_gate)
    return out
